# revision 12
# baseline (speedup 1.0000x reference)
"""Multi-head attention on 8 TRN2 NeuronCores (data/head-parallel).

Problem: B=4 H=16 S=2048 D=64 fp32 attention, out = softmax(Q K^T / sqrt(D)) V.
B*H = 64 (batch, head) pairs are sharded 8-per-core; each core runs the same
NEFF over its own 8 heads, no collectives.

The baseline bf16 kernel was jointly PE-bound (~260us busy) and ACT-bound
(~261us: 33.5M exps at 128 lanes/cycle).  Structure of this version:

  - All matmuls are bf16 with 128x128 stationaries, which keeps Fast Weight
    Load eligible: LDWEIGHTS goes to the background weight buffer and hides
    under in-flight matmuls.  (fp8 DoubleRow and PE row-tiling were both
    measured slower: their weight loads are foreground-only and serialize.)
  - Score and P@V matmuls are 512-column (one PSUM bank), interleaved one
    QK : one P@V of the previous chunk, so the PE never waits on the exp
    engines: four single-bank score tiles rotate against a double-buffered
    P@V accumulator (8 PSUM banks exactly).
  - exp splits across engines at half-tile granularity: the low 512 q of
    every score tile gets the ACT hw exp, the high 512 q get a Schraudolph
    bit-hack on the otherwise-idle DVE -- E = bitcast_bf16(int16(A*s + B)),
    one tensor_scalar into an int16 view of the same bf16 E tile (~1.8%
    rms sawtooth on those columns; softmax renormalization cancels the
    mean; measured end-to-end ~1.1% vs the 2% budget).
  - The output [d,q]->[q,d] transpose runs on the DMA XBAR
    (dma_start_transpose of a bf16 [80,1024] tile), not the PE.
  - Normalization: denominators ride a ones column in V through the P@V
    matmul; reciprocals on DVE; the per-row scales on GpSimd (SBUF-resident
    after the XBAR transpose); PSUM->SBUF copies on ACT.
  - Software pipeline: iteration g runs QK+exp of chunk g interleaved with
    P@V of chunk g-1 on the PE, then the copy of g-1 (ACT) and the
    normalize+store of g-2 (DVE reciprocal, GpSimd scale, Sync DMA).
"""

import math
from contextlib import ExitStack

import ml_dtypes
import numpy as np

import concourse.bass as bass
import concourse.bacc as bacc
import concourse.tile as tile
import concourse.mybir as mybir
from concourse.bass_utils import run_bass_kernel_spmd

B, H, S, D = 4, 16, 2048, 64
N_CORES = 8
HPC = B * H // N_CORES     # heads per core
ST = S // 128              # 16 k-tiles of 128
QCHUNK = 1024              # q processed in chunks (PSUM budget)
NQ = S // QCHUNK
NJ = QCHUNK // 128         # 128-q output groups per chunk
DT = mybir.dt

# Schraudolph int16/bf16 exp: E = bitcast_bf16(int16(A*s + B)), trunc-calibrated
SCHRAUD_A = 128.0 / math.log(2.0)                # * scale at runtime
SCHRAUD_B = 127.0 * 128.0 + 0.5 - 0.0430 * 128.0

_BUILT = {}


class _Bacc(bacc.Bacc):
    """Bacc with the move-matmul-waits-to-ldweights pass disabled: keeping
    waits on the matmul (not its LDWEIGHTS) lets the PE queue pull weight
    loads ahead of in-flight matmuls, hiding the LDW cost."""

    def move_matmul_waits_to_ldweights(self):
        pass


def _load_head(nc, stage, qt_d, kt_d, vp_d, h, first):
    qt = stage.tile([128, S], DT.bfloat16, tag="qt")
    kt = stage.tile([128, S], DT.bfloat16, tag="kt")
    vp = stage.tile([128, ST, 128], DT.bfloat16, tag="vp")
    if first:
        # Cold start: the first matmuls need kt[:, :128] and qt[:, :1024];
        # fetch those first on the idle Sync/Scalar HWDGEs so the PE starts
        # within a couple of microseconds.
        nc.sync.dma_start(out=kt[:, 0:512], in_=kt_d[h][:, 0:512])
        nc.scalar.dma_start(out=qt[:, 0:1024], in_=qt_d[h][:, 0:1024])
        nc.sync.dma_start(out=kt[:, 512:2048], in_=kt_d[h][:, 512:2048])
        nc.scalar.dma_start(out=qt[:, 1024:2048], in_=qt_d[h][:, 1024:2048])
    else:
        for j in range(2):
            half = slice(j * (S // 2), (j + 1) * (S // 2))
            nc.gpsimd.dma_start(out=kt[:, half], in_=kt_d[h][:, half])
            nc.gpsimd.dma_start(out=qt[:, half], in_=qt_d[h][:, half])
    vp_v = vp_d[h].rearrange("(t p) e -> p t e", p=128)
    for j in range(2):
        sl = slice(8 * j, 8 * j + 8)
        nc.gpsimd.dma_start(out=vp[:, sl, :], in_=vp_v[:, sl, :])
    return qt, kt, vp


def build_graph(scale: float, heads: int = HPC):
    nc = _Bacc("TRN2", target_bir_lowering=False, debug=False,
               num_devices=N_CORES)
    qt_d = nc.dram_tensor("QT", [heads, 128, S], DT.bfloat16,
                          kind="ExternalInput").ap()
    kt_d = nc.dram_tensor("KT", [heads, 128, S], DT.bfloat16,
                          kind="ExternalInput").ap()
    vp_d = nc.dram_tensor("VP", [heads, S, 128], DT.bfloat16,
                          kind="ExternalInput").ap()
    o_d = nc.dram_tensor("out", [heads, S, D], DT.float32,
                         kind="ExternalOutput").ap()

    a_s = float(scale) * SCHRAUD_A

    with tile.TileContext(nc) as tc, ExitStack() as ctx:
        stage = ctx.enter_context(tc.tile_pool(name="stage", bufs=3))
        epool = ctx.enter_context(tc.tile_pool(name="epool", bufs=2))
        spool = ctx.enter_context(tc.tile_pool(name="spool", bufs=2))
        trp = ctx.enter_context(tc.tile_pool(name="trp", bufs=2))
        outp = ctx.enter_context(tc.tile_pool(name="outp", bufs=2))
        recp = ctx.enter_context(tc.tile_pool(name="recp", bufs=2))
        ps_st = ctx.enter_context(tc.tile_pool(name="ps_st", bufs=4, space="PSUM"))
        ps_ot = ctx.enter_context(tc.tile_pool(name="ps_ot", bufs=2, space="PSUM"))

        gs = [(h, c) for h in range(heads) for c in range(NQ)]
        head_tiles = {}
        state = {}   # iteration -> dict(ets, vp, ot, otr, h, c)

        head_tiles[0] = _load_head(nc, stage, qt_d, kt_d, vp_d, 0, True)

        for i in range(len(gs) + 2):
            cur = gs[i] if i < len(gs) else None
            if cur is not None:
                h, c = cur
                if c == 0 and h + 1 < heads:
                    head_tiles[h + 1] = _load_head(nc, stage, qt_d, kt_d,
                                                   vp_d, h + 1, False)
                qt, kt, vp = head_tiles[h]
                q0 = c * QCHUNK
                ets = []
                state[i] = {"h": h, "c": c, "vp": vp, "ets": ets}
            prev = state.get(i - 1)
            fin = state.pop(i - 2, None)

            # Finalize state for iteration i-2, spread through the t-loop so
            # the DVE never bursts and delays exp work at chunk boundaries.
            if fin is not None:
                fin["rec"] = recp.tile([128, NJ], DT.float32, tag="rec",
                                       name="rec")
                fin["outst"] = outp.tile([128, NJ, D], DT.float32,
                                         tag="outst", name="outst")

            for t in range(ST):
                if prev is not None and t == 0:
                    prev["ot"] = ps_ot.tile([128, QCHUNK], DT.float32,
                                            tag="ot", name="ot")
                st2 = None
                if cur is not None:
                    # Two matmuls share each kt stationary (one-per-stationary
                    # streams are limited by the background weight-load rate);
                    # each lands in its own single-bank PSUM tile so one
                    # engine alone frees it.
                    st2 = [ps_st.tile([128, 512], DT.float32, tag="st",
                                      name="st") for _ in range(2)]
                    for n in range(2):
                        nc.tensor.matmul(
                            st2[n],
                            lhsT=kt[:, t * 128:(t + 1) * 128],
                            rhs=qt[:, q0 + n * 512:q0 + (n + 1) * 512],
                            start=True, stop=True)
                if prev is not None:
                    for n in range(2):
                        osl = slice(n * 512, (n + 1) * 512)
                        nc.tensor.matmul(
                            prev["ot"][:, osl],
                            lhsT=prev["vp"][:, t, :],
                            rhs=prev["ets"][t][:, osl],
                            start=(t == 0), stop=(t == ST - 1))
                if st2 is not None:
                    et = epool.tile([128, QCHUNK], DT.bfloat16, tag=f"et{t}")
                    ets.append(et)
                    # low half -> ACT hw exp; high half -> DVE Schraudolph
                    nc.scalar.activation(
                        out=et[:, 0:512], in_=st2[0],
                        func=mybir.ActivationFunctionType.Exp, scale=scale)
                    nc.vector.tensor_scalar(
                        et.bitcast(DT.int16)[:, 512:1024],
                        st2[1], a_s, SCHRAUD_B,
                        mybir.AluOpType.mult, mybir.AluOpType.add)
                if fin is not None:
                    if t == 2:
                        nc.vector.reciprocal(out=fin["rec"],
                                             in_=fin["otr"][:, :, D])
                    elif 3 <= t <= 10:
                        j = t - 3
                        nc.vector.tensor_scalar(
                            fin["outst"][:, j, :], fin["otr"][:, j, 0:D],
                            fin["rec"][:, j:j + 1],
                            None, mybir.AluOpType.mult)
                    elif t == 11:
                        o_v = o_d[fin["h"],
                                  fin["c"] * QCHUNK:(fin["c"] + 1) * QCHUNK, :]
                        o_v = o_v.rearrange("(r p) d -> p r d", p=128)
                        nc.sync.dma_start(out=o_v, in_=fin["outst"])

            if prev is not None:
                # PSUM -> SBUF as bf16 (ACT), then [80,1024] -> [1024,80] on
                # the DMA XBAR, in two halves to shorten the drain chain.
                # Rows 65..79 are the zero-padded V columns.
                ots = spool.tile([80, QCHUNK], DT.bfloat16, tag="ots")
                otr = trp.tile([128, NJ, 80], DT.bfloat16, tag="otr")
                for nh in range(2):
                    osl = slice(nh * 512, (nh + 1) * 512)
                    jsl = slice(nh * (NJ // 2), (nh + 1) * (NJ // 2))
                    nc.scalar.copy(out=ots[:, osl], in_=prev["ot"][0:80, osl])
                    nc.sync.dma_start_transpose(out=otr[:, jsl, :],
                                                in_=ots[:, osl])
                prev["otr"] = otr

    nc.compile()
    return nc


def _get_nc(scale: float):
    key = round(float(scale), 9)
    if key not in _BUILT:
        _BUILT[key] = build_graph(float(scale))
    return _BUILT[key]


def shard_inputs(Q, K, V):
    """Host-side prep: shard heads across cores, pre-transpose Q/K to [D,S]
    bf16 (zero-padded to 128 partitions), append a ones column to V (bf16)."""
    bf16 = ml_dtypes.bfloat16
    BH = B * H
    qs = np.asarray(Q, dtype=np.float32).reshape(BH, S, D)
    ks = np.asarray(K, dtype=np.float32).reshape(BH, S, D)
    vs = np.asarray(V, dtype=np.float32).reshape(BH, S, D)
    qt = np.zeros((BH, 128, S), dtype=bf16)
    kt = np.zeros((BH, 128, S), dtype=bf16)
    qt[:, :D, :] = qs.transpose(0, 2, 1).astype(bf16)
    kt[:, :D, :] = ks.transpose(0, 2, 1).astype(bf16)
    vp = np.zeros((BH, S, 128), dtype=bf16)
    vp[:, :, :D] = vs.astype(bf16)
    vp[:, :, D] = np.float32(1.0)

    in_maps = []
    for c in range(N_CORES):
        sl = slice(c * HPC, (c + 1) * HPC)
        in_maps.append({
            "QT": np.ascontiguousarray(qt[sl]),
            "KT": np.ascontiguousarray(kt[sl]),
            "VP": np.ascontiguousarray(vp[sl]),
        })
    return in_maps


def kernel(Q, K, V, d_k, **run_kwargs):
    scale = 1.0 / math.sqrt(float(d_k))
    nc = _get_nc(scale)
    in_maps = shard_inputs(Q, K, V)
    res = run_bass_kernel_spmd(nc, in_maps, core_ids=list(range(N_CORES)),
                               **run_kwargs)
    out = np.concatenate([r["out"] for r in res.results], axis=0)
    out = out.reshape(B, H, S, D).astype(np.float32)
    kernel.last_results = res
    return out


# revision 13
# speedup vs baseline: 1.4357x; 1.4357x over previous
"""Multi-head attention on 8 TRN2 NeuronCores (data/head-parallel).

Problem: B=4 H=16 S=2048 D=64 fp32 attention, out = softmax(Q K^T / sqrt(D)) V.
B*H = 64 (batch, head) pairs are sharded 8-per-core; each core runs the same
NEFF over its own 8 heads, no collectives.

The baseline bf16 kernel was jointly PE-bound (~260us busy) and ACT-bound
(~261us: 33.5M exps at 128 lanes/cycle).  Structure of this version:

  - All matmuls are bf16 with 128x128 stationaries, which keeps Fast Weight
    Load eligible: LDWEIGHTS goes to the background weight buffer and hides
    under in-flight matmuls.  (fp8 DoubleRow and PE row-tiling were both
    measured slower: their weight loads are foreground-only and serialize.)
  - Score and P@V matmuls are 512-column (one PSUM bank), interleaved one
    QK : one P@V of the previous chunk, so the PE never waits on the exp
    engines: four single-bank score tiles rotate against a double-buffered
    P@V accumulator (8 PSUM banks exactly).
  - exp splits across engines at half-tile granularity: the low 512 q of
    every score tile gets the ACT hw exp, the high 512 q get a Schraudolph
    bit-hack on the otherwise-idle DVE -- E = bitcast_bf16(int16(A*s + B)),
    one tensor_scalar into an int16 view of the same bf16 E tile (~1.8%
    rms sawtooth on those columns; softmax renormalization cancels the
    mean; measured end-to-end ~1.1% vs the 2% budget).
  - The output [d,q]->[q,d] transpose runs on the DMA XBAR
    (dma_start_transpose of a bf16 [80,1024] tile), not the PE.
  - Normalization: denominators ride a ones column in V through the P@V
    matmul; reciprocals on DVE; the per-row scales on GpSimd (SBUF-resident
    after the XBAR transpose); PSUM->SBUF copies on ACT.
  - Software pipeline: iteration g runs QK+exp of chunk g interleaved with
    P@V of chunk g-1 on the PE, then the copy of g-1 (ACT) and the
    normalize+store of g-2 (DVE reciprocal, GpSimd scale, Sync DMA).
"""

import math
from contextlib import ExitStack

import ml_dtypes
import numpy as np

import concourse.bass as bass
import concourse.bacc as bacc
import concourse.tile as tile
import concourse.mybir as mybir
from concourse.bass_utils import run_bass_kernel_spmd

B, H, S, D = 4, 16, 2048, 64
N_CORES = 8
HPC = B * H // N_CORES     # heads per core
ST = S // 128              # 16 k-tiles of 128
QCHUNK = 1024              # q processed in chunks (PSUM budget)
NQ = S // QCHUNK
NJ = QCHUNK // 128         # 128-q output groups per chunk
DT = mybir.dt

# Schraudolph int16/bf16 exp: E = bitcast_bf16(int16(A*s + B)), trunc-calibrated
SCHRAUD_A = 128.0 / math.log(2.0)                # * scale at runtime
SCHRAUD_B = 127.0 * 128.0 + 0.5 - 0.0430 * 128.0

_BUILT = {}


class _Bacc(bacc.Bacc):
    """Bacc with the move-matmul-waits-to-ldweights pass disabled: keeping
    waits on the matmul (not its LDWEIGHTS) lets the PE queue pull weight
    loads ahead of in-flight matmuls, hiding the LDW cost."""

    def move_matmul_waits_to_ldweights(self):
        pass


def _load_head(nc, stage, qt_d, kt_d, vp_d, h, first):
    qt = stage.tile([128, S], DT.bfloat16, tag="qt")
    kt = stage.tile([128, S], DT.bfloat16, tag="kt")
    vp = stage.tile([128, S], DT.bfloat16, tag="vp")
    if first:
        # Cold start: the first matmuls need kt[:, :128] and qt[:, :1024];
        # fetch those first on the idle Sync/Scalar HWDGEs so the PE starts
        # within a couple of microseconds.
        nc.sync.dma_start(out=kt[:, 0:512], in_=kt_d[h][:, 0:512])
        nc.scalar.dma_start(out=qt[:, 0:1024], in_=qt_d[h][:, 0:1024])
        nc.sync.dma_start(out=kt[:, 512:2048], in_=kt_d[h][:, 512:2048])
        nc.scalar.dma_start(out=qt[:, 1024:2048], in_=qt_d[h][:, 1024:2048])
    else:
        for j in range(2):
            half = slice(j * (S // 2), (j + 1) * (S // 2))
            nc.gpsimd.dma_start(out=kt[:, half], in_=kt_d[h][:, half])
            nc.gpsimd.dma_start(out=qt[:, half], in_=qt_d[h][:, half])
    for j in range(2):
        half = slice(j * (S // 2), (j + 1) * (S // 2))
        nc.gpsimd.dma_start(out=vp[:, half], in_=vp_d[h][:, half])
    return qt, kt, vp


def build_graph(scale: float, heads: int = HPC):
    nc = _Bacc("TRN2", target_bir_lowering=False, debug=False,
               num_devices=N_CORES)
    qt_d = nc.dram_tensor("QT", [heads, 128, S], DT.bfloat16,
                          kind="ExternalInput").ap()
    kt_d = nc.dram_tensor("KT", [heads, 128, S], DT.bfloat16,
                          kind="ExternalInput").ap()
    vp_d = nc.dram_tensor("VP", [heads, 128, S], DT.bfloat16,
                          kind="ExternalInput").ap()
    o_d = nc.dram_tensor("out", [heads, S, D], DT.float32,
                         kind="ExternalOutput").ap()

    a_s = float(scale) * SCHRAUD_A

    with tile.TileContext(nc) as tc, ExitStack() as ctx:
        stage = ctx.enter_context(tc.tile_pool(name="stage", bufs=3))
        epool = ctx.enter_context(tc.tile_pool(name="epool", bufs=2))
        spool = ctx.enter_context(tc.tile_pool(name="spool", bufs=2))
        trp = ctx.enter_context(tc.tile_pool(name="trp", bufs=2))
        outp = ctx.enter_context(tc.tile_pool(name="outp", bufs=2))
        recp = ctx.enter_context(tc.tile_pool(name="recp", bufs=2))
        ps_st = ctx.enter_context(tc.tile_pool(name="ps_st", bufs=4, space="PSUM"))
        ps_ot = ctx.enter_context(tc.tile_pool(name="ps_ot", bufs=2, space="PSUM"))

        gs = [(h, c) for h in range(heads) for c in range(NQ)]
        head_tiles = {}
        state = {}   # iteration -> dict(ets, vp, ot, otr, h, c)

        head_tiles[0] = _load_head(nc, stage, qt_d, kt_d, vp_d, 0, True)

        for i in range(len(gs) + 2):
            cur = gs[i] if i < len(gs) else None
            if cur is not None:
                h, c = cur
                if c == NQ - 1 and h + 1 < heads:
                    head_tiles[h + 1] = _load_head(nc, stage, qt_d, kt_d,
                                                   vp_d, h + 1, False)
                qt, kt, vp = head_tiles[h]
                q0 = c * QCHUNK
                ets = []
                state[i] = {"h": h, "c": c, "vp": vp, "ets": ets}
            prev = state.get(i - 1)
            fin = state.pop(i - 2, None)

            # Finalize state for iteration i-2, spread through the t-loop so
            # the DVE never bursts and delays exp work at chunk boundaries.
            if fin is not None:
                fin["rec"] = recp.tile([128, NJ], DT.float32, tag="rec",
                                       name="rec")
                fin["outst"] = outp.tile([128, NJ, D], DT.float32,
                                         tag="outst", name="outst")

            for t in range(ST):
                if prev is not None and t == 0:
                    prev["ot"] = ps_ot.tile([128, QCHUNK], DT.float32,
                                            tag="ot", name="ot")
                st2 = None
                if cur is not None:
                    # Two matmuls share each kt stationary (one-per-stationary
                    # streams are limited by the background weight-load rate);
                    # each lands in its own single-bank PSUM tile so one
                    # engine alone frees it.
                    st2 = [ps_st.tile([128, 512], DT.float32, tag="st",
                                      name="st") for _ in range(2)]
                    for n in range(2):
                        nc.tensor.matmul(
                            st2[n],
                            lhsT=kt[:, t * 128:(t + 1) * 128],
                            rhs=qt[:, q0 + n * 512:q0 + (n + 1) * 512],
                            start=True, stop=True)
                if prev is not None:
                    for n in range(2):
                        osl = slice(n * 512, (n + 1) * 512)
                        nc.tensor.matmul(
                            prev["ot"][:, osl],
                            lhsT=prev["vp"][:, t * 128:(t + 1) * 128],
                            rhs=prev["ets"][t][:, osl],
                            start=(t == 0), stop=(t == ST - 1))
                if st2 is not None:
                    et = epool.tile([128, QCHUNK], DT.bfloat16, tag=f"et{t}")
                    ets.append(et)
                    # low half -> ACT hw exp; high half -> DVE Schraudolph
                    nc.scalar.activation(
                        out=et[:, 0:512], in_=st2[0],
                        func=mybir.ActivationFunctionType.Exp, scale=scale)
                    nc.vector.tensor_scalar(
                        et.bitcast(DT.int16)[:, 512:1024],
                        st2[1], a_s, SCHRAUD_B,
                        mybir.AluOpType.mult, mybir.AluOpType.add)
                if fin is not None:
                    if t == 2:
                        nc.vector.reciprocal(out=fin["rec"],
                                             in_=fin["otr"][:, :, D])
                    elif 3 <= t <= 10:
                        j = t - 3
                        nc.vector.tensor_scalar(
                            fin["outst"][:, j, :], fin["otr"][:, j, 0:D],
                            fin["rec"][:, j:j + 1],
                            None, mybir.AluOpType.mult)
                    elif t == 11:
                        o_v = o_d[fin["h"],
                                  fin["c"] * QCHUNK:(fin["c"] + 1) * QCHUNK, :]
                        o_v = o_v.rearrange("(r p) d -> p r d", p=128)
                        nc.sync.dma_start(out=o_v, in_=fin["outst"])

            if prev is not None:
                # PSUM -> SBUF as bf16 (ACT), then [80,1024] -> [1024,80] on
                # the DMA XBAR.  Rows 65..79 are the zero-padded V columns.
                ots = spool.tile([80, QCHUNK], DT.bfloat16, tag="ots")
                nc.scalar.copy(out=ots, in_=prev["ot"][0:80, :])
                otr = trp.tile([128, NJ, 80], DT.bfloat16, tag="otr")
                nc.sync.dma_start_transpose(out=otr, in_=ots)
                prev["otr"] = otr

    nc.compile()
    return nc


def _get_nc(scale: float):
    key = round(float(scale), 9)
    if key not in _BUILT:
        _BUILT[key] = build_graph(float(scale))
    return _BUILT[key]


def shard_inputs(Q, K, V):
    """Host-side prep: shard heads across cores, pre-transpose Q/K to [D,S]
    bf16 (zero-padded to 128 partitions), append a ones column to V (bf16)."""
    bf16 = ml_dtypes.bfloat16
    BH = B * H
    qs = np.asarray(Q, dtype=np.float32).reshape(BH, S, D)
    ks = np.asarray(K, dtype=np.float32).reshape(BH, S, D)
    vs = np.asarray(V, dtype=np.float32).reshape(BH, S, D)
    qt = np.zeros((BH, 128, S), dtype=bf16)
    kt = np.zeros((BH, 128, S), dtype=bf16)
    qt[:, :D, :] = qs.transpose(0, 2, 1).astype(bf16)
    kt[:, :D, :] = ks.transpose(0, 2, 1).astype(bf16)
    vpo = np.zeros((BH, S, 128), dtype=bf16)
    vpo[:, :, :D] = vs.astype(bf16)
    vpo[:, :, D] = np.float32(1.0)
    # partition-major: vp[h, p, t*128+e] = V'[t*128+p, e] -- contiguous DMA
    vp = np.ascontiguousarray(
        vpo.reshape(BH, ST, 128, 128).transpose(0, 2, 1, 3).reshape(BH, 128, S))

    in_maps = []
    for c in range(N_CORES):
        sl = slice(c * HPC, (c + 1) * HPC)
        in_maps.append({
            "QT": np.ascontiguousarray(qt[sl]),
            "KT": np.ascontiguousarray(kt[sl]),
            "VP": np.ascontiguousarray(vp[sl]),
        })
    return in_maps


def kernel(Q, K, V, d_k, **run_kwargs):
    scale = 1.0 / math.sqrt(float(d_k))
    nc = _get_nc(scale)
    in_maps = shard_inputs(Q, K, V)
    res = run_bass_kernel_spmd(nc, in_maps, core_ids=list(range(N_CORES)),
                               **run_kwargs)
    out = np.concatenate([r["out"] for r in res.results], axis=0)
    out = out.reshape(B, H, S, D).astype(np.float32)
    kernel.last_results = res
    return out


# revision 14
# speedup vs baseline: 1.4485x; 1.0089x over previous
"""Multi-head attention on 8 TRN2 NeuronCores (data/head-parallel).

Problem: B=4 H=16 S=2048 D=64 fp32 attention, out = softmax(Q K^T / sqrt(D)) V.
B*H = 64 (batch, head) pairs are sharded 8-per-core; each core runs the same
NEFF over its own 8 heads, no collectives.

The baseline bf16 kernel was jointly PE-bound (~260us busy) and ACT-bound
(~261us: 33.5M exps at 128 lanes/cycle).  Structure of this version:

  - All matmuls are bf16 with 128x128 stationaries, which keeps Fast Weight
    Load eligible: LDWEIGHTS goes to the background weight buffer and hides
    under in-flight matmuls.  (fp8 DoubleRow and PE row-tiling were both
    measured slower: their weight loads are foreground-only and serialize.)
  - Score and P@V matmuls are 512-column (one PSUM bank), interleaved one
    QK : one P@V of the previous chunk, so the PE never waits on the exp
    engines: four single-bank score tiles rotate against a double-buffered
    P@V accumulator (8 PSUM banks exactly).
  - exp splits across engines at half-tile granularity: the low 512 q of
    every score tile gets the ACT hw exp, the high 512 q get a Schraudolph
    bit-hack on the otherwise-idle DVE -- E = bitcast_bf16(int16(A*s + B)),
    one tensor_scalar into an int16 view of the same bf16 E tile (~1.8%
    rms sawtooth on those columns; softmax renormalization cancels the
    mean; measured end-to-end ~1.1% vs the 2% budget).
  - The output [d,q]->[q,d] transpose runs on the DMA XBAR
    (dma_start_transpose of a bf16 [80,1024] tile), not the PE.
  - Normalization: denominators ride a ones column in V through the P@V
    matmul; reciprocals on DVE; the per-row scales on GpSimd (SBUF-resident
    after the XBAR transpose); PSUM->SBUF copies on ACT.
  - Software pipeline: iteration g runs QK+exp of chunk g interleaved with
    P@V of chunk g-1 on the PE, then the copy of g-1 (ACT) and the
    normalize+store of g-2 (DVE reciprocal, GpSimd scale, Sync DMA).
"""

import math
from contextlib import ExitStack

import ml_dtypes
import numpy as np

import concourse.bass as bass
import concourse.bacc as bacc
import concourse.tile as tile
import concourse.mybir as mybir
from concourse.bass_utils import run_bass_kernel_spmd

B, H, S, D = 4, 16, 2048, 64
N_CORES = 8
HPC = B * H // N_CORES     # heads per core
ST = S // 128              # 16 k-tiles of 128
QCHUNK = 1024              # q processed in chunks (PSUM budget)
NQ = S // QCHUNK
NJ = QCHUNK // 128         # 128-q output groups per chunk
DT = mybir.dt

# Schraudolph int16/bf16 exp: E = bitcast_bf16(int16(A*s + B)), trunc-calibrated
SCHRAUD_A = 128.0 / math.log(2.0)                # * scale at runtime
SCHRAUD_B = 127.0 * 128.0 + 0.5 - 0.0430 * 128.0

_BUILT = {}


class _Bacc(bacc.Bacc):
    """Bacc with the move-matmul-waits-to-ldweights pass disabled: keeping
    waits on the matmul (not its LDWEIGHTS) lets the PE queue pull weight
    loads ahead of in-flight matmuls, hiding the LDW cost."""

    def move_matmul_waits_to_ldweights(self):
        pass


def _load_head(nc, stage, qt_d, kt_d, vp_d, h, first):
    qt = stage.tile([128, S], DT.bfloat16, tag="qt")
    kt = stage.tile([128, S], DT.bfloat16, tag="kt")
    vp = stage.tile([128, S], DT.bfloat16, tag="vp")
    if first:
        # Cold start: the first matmuls need kt[:, :128] and qt[:, :1024];
        # fetch those first on the idle Sync/Scalar HWDGEs so the PE starts
        # within a couple of microseconds.
        nc.sync.dma_start(out=kt[:, 0:512], in_=kt_d[h][:, 0:512])
        nc.scalar.dma_start(out=qt[:, 0:1024], in_=qt_d[h][:, 0:1024])
        nc.sync.dma_start(out=kt[:, 512:2048], in_=kt_d[h][:, 512:2048])
        nc.scalar.dma_start(out=qt[:, 1024:2048], in_=qt_d[h][:, 1024:2048])
    else:
        for j in range(2):
            half = slice(j * (S // 2), (j + 1) * (S // 2))
            nc.gpsimd.dma_start(out=kt[:, half], in_=kt_d[h][:, half])
            nc.gpsimd.dma_start(out=qt[:, half], in_=qt_d[h][:, half])
    for j in range(2):
        half = slice(j * (S // 2), (j + 1) * (S // 2))
        nc.gpsimd.dma_start(out=vp[:, half], in_=vp_d[h][:, half])
    return qt, kt, vp


def build_graph(scale: float, heads: int = HPC):
    nc = _Bacc("TRN2", target_bir_lowering=False, debug=False,
               num_devices=N_CORES)
    qt_d = nc.dram_tensor("QT", [heads, 128, S], DT.bfloat16,
                          kind="ExternalInput").ap()
    kt_d = nc.dram_tensor("KT", [heads, 128, S], DT.bfloat16,
                          kind="ExternalInput").ap()
    vp_d = nc.dram_tensor("VP", [heads, 128, S], DT.bfloat16,
                          kind="ExternalInput").ap()
    o_d = nc.dram_tensor("out", [heads, S, D], DT.float32,
                         kind="ExternalOutput").ap()

    a_s = float(scale) * SCHRAUD_A

    with tile.TileContext(nc) as tc, ExitStack() as ctx:
        stage = ctx.enter_context(tc.tile_pool(name="stage", bufs=4))
        epool = ctx.enter_context(tc.tile_pool(name="epool", bufs=2))
        spool = ctx.enter_context(tc.tile_pool(name="spool", bufs=3))
        trp = ctx.enter_context(tc.tile_pool(name="trp", bufs=3))
        outp = ctx.enter_context(tc.tile_pool(name="outp", bufs=2))
        recp = ctx.enter_context(tc.tile_pool(name="recp", bufs=2))
        ps_st = ctx.enter_context(tc.tile_pool(name="ps_st", bufs=4, space="PSUM"))
        ps_ot = ctx.enter_context(tc.tile_pool(name="ps_ot", bufs=2, space="PSUM"))

        gs = [(h, c) for h in range(heads) for c in range(NQ)]
        head_tiles = {}
        state = {}   # iteration -> dict(ets, vp, ot, otr, h, c)

        head_tiles[0] = _load_head(nc, stage, qt_d, kt_d, vp_d, 0, True)

        for i in range(len(gs) + 2):
            cur = gs[i] if i < len(gs) else None
            if cur is not None:
                h, c = cur
                if c == NQ - 1 and h + 1 < heads:
                    head_tiles[h + 1] = _load_head(nc, stage, qt_d, kt_d,
                                                   vp_d, h + 1, False)
                qt, kt, vp = head_tiles[h]
                q0 = c * QCHUNK
                ets = []
                state[i] = {"h": h, "c": c, "vp": vp, "ets": ets}
            prev = state.get(i - 1)
            fin = state.pop(i - 2, None)

            # Finalize state for iteration i-2, spread through the t-loop so
            # the DVE never bursts and delays exp work at chunk boundaries.
            if fin is not None:
                fin["rec"] = recp.tile([128, NJ], DT.float32, tag="rec",
                                       name="rec")
                fin["outst"] = outp.tile([128, NJ, D], DT.float32,
                                         tag="outst", name="outst")

            for t in range(ST):
                if prev is not None and t == 0:
                    prev["ot"] = ps_ot.tile([128, QCHUNK], DT.float32,
                                            tag="ot", name="ot")
                st2 = None
                if cur is not None:
                    # Two matmuls share each kt stationary (one-per-stationary
                    # streams are limited by the background weight-load rate);
                    # each lands in its own single-bank PSUM tile so one
                    # engine alone frees it.
                    st2 = [ps_st.tile([128, 512], DT.float32, tag="st",
                                      name="st") for _ in range(2)]
                    for n in range(2):
                        nc.tensor.matmul(
                            st2[n],
                            lhsT=kt[:, t * 128:(t + 1) * 128],
                            rhs=qt[:, q0 + n * 512:q0 + (n + 1) * 512],
                            start=True, stop=True)
                if prev is not None:
                    for n in range(2):
                        osl = slice(n * 512, (n + 1) * 512)
                        nc.tensor.matmul(
                            prev["ot"][:, osl],
                            lhsT=prev["vp"][:, t * 128:(t + 1) * 128],
                            rhs=prev["ets"][t][:, osl],
                            start=(t == 0), stop=(t == ST - 1))
                if st2 is not None:
                    et = epool.tile([128, QCHUNK], DT.bfloat16, tag=f"et{t}")
                    ets.append(et)
                    # low half -> ACT hw exp; high half -> DVE Schraudolph
                    nc.scalar.activation(
                        out=et[:, 0:512], in_=st2[0],
                        func=mybir.ActivationFunctionType.Exp, scale=scale)
                    nc.vector.tensor_scalar(
                        et.bitcast(DT.int16)[:, 512:1024],
                        st2[1], a_s, SCHRAUD_B,
                        mybir.AluOpType.mult, mybir.AluOpType.add)
                if fin is not None:
                    if t == 6:
                        nc.vector.reciprocal(out=fin["rec"],
                                             in_=fin["otr"][:, :, D])
                    elif 7 <= t <= 14:
                        j = t - 7
                        nc.vector.tensor_scalar(
                            fin["outst"][:, j, :], fin["otr"][:, j, 0:D],
                            fin["rec"][:, j:j + 1],
                            None, mybir.AluOpType.mult)
                    elif t == 15:
                        o_v = o_d[fin["h"],
                                  fin["c"] * QCHUNK:(fin["c"] + 1) * QCHUNK, :]
                        o_v = o_v.rearrange("(r p) d -> p r d", p=128)
                        nc.sync.dma_start(out=o_v, in_=fin["outst"])

            if prev is not None:
                # PSUM -> SBUF as bf16 (ACT), then [80,1024] -> [1024,80] on
                # the DMA XBAR.  Rows 65..79 are the zero-padded V columns.
                ots = spool.tile([80, QCHUNK], DT.bfloat16, tag="ots")
                nc.scalar.copy(out=ots, in_=prev["ot"][0:80, :])
                otr = trp.tile([128, NJ, 80], DT.bfloat16, tag="otr")
                nc.sync.dma_start_transpose(out=otr, in_=ots)
                prev["otr"] = otr

    nc.compile()
    return nc


def _get_nc(scale: float):
    key = round(float(scale), 9)
    if key not in _BUILT:
        _BUILT[key] = build_graph(float(scale))
    return _BUILT[key]


def shard_inputs(Q, K, V):
    """Host-side prep: shard heads across cores, pre-transpose Q/K to [D,S]
    bf16 (zero-padded to 128 partitions), append a ones column to V (bf16)."""
    bf16 = ml_dtypes.bfloat16
    BH = B * H
    qs = np.asarray(Q, dtype=np.float32).reshape(BH, S, D)
    ks = np.asarray(K, dtype=np.float32).reshape(BH, S, D)
    vs = np.asarray(V, dtype=np.float32).reshape(BH, S, D)
    qt = np.zeros((BH, 128, S), dtype=bf16)
    kt = np.zeros((BH, 128, S), dtype=bf16)
    qt[:, :D, :] = qs.transpose(0, 2, 1).astype(bf16)
    kt[:, :D, :] = ks.transpose(0, 2, 1).astype(bf16)
    vpo = np.zeros((BH, S, 128), dtype=bf16)
    vpo[:, :, :D] = vs.astype(bf16)
    vpo[:, :, D] = np.float32(1.0)
    # partition-major: vp[h, p, t*128+e] = V'[t*128+p, e] -- contiguous DMA
    vp = np.ascontiguousarray(
        vpo.reshape(BH, ST, 128, 128).transpose(0, 2, 1, 3).reshape(BH, 128, S))

    in_maps = []
    for c in range(N_CORES):
        sl = slice(c * HPC, (c + 1) * HPC)
        in_maps.append({
            "QT": np.ascontiguousarray(qt[sl]),
            "KT": np.ascontiguousarray(kt[sl]),
            "VP": np.ascontiguousarray(vp[sl]),
        })
    return in_maps


def kernel(Q, K, V, d_k, **run_kwargs):
    scale = 1.0 / math.sqrt(float(d_k))
    nc = _get_nc(scale)
    in_maps = shard_inputs(Q, K, V)
    res = run_bass_kernel_spmd(nc, in_maps, core_ids=list(range(N_CORES)),
                               **run_kwargs)
    out = np.concatenate([r["out"] for r in res.results], axis=0)
    out = out.reshape(B, H, S, D).astype(np.float32)
    kernel.last_results = res
    return out
